# revision 1
# baseline (speedup 1.0000x reference)
"""Causal multi-head self-attention on 8 Trainium2 NeuronCores.

Problem: X[4, 2048, 1024] fp32, W_Q/W_K/W_V/W_O [1024, 1024] fp32,
16 heads x 64 dims, causal softmax attention + output projection.

Sharding: core c handles batch b = c//2 and head-group g = c%2
(heads g*8..g*8+8, i.e. 512 of the 1024 channels).  Each core computes
its 8 heads' Q/K/V projections, causal attention, and a partial output
projection against W_O[:, g*512:(g+1)*512]; the host sums the two
partial outputs per batch (the "all-reduce after W_O" step).

Device kernel layout notes:
 - ALL matmuls use [128, <=128] stationaries so the PE array stays in
   128x128 tiling mode for the whole kernel (mode switches drain the
   array).  Score stationaries are zero-padded: kt_z[:, h2] holds head
   h2's K^T in partition rows h2*64..h2*64+63 and zeros elsewhere, so a
   full-128-contraction matmul against the stacked Q produces that
   head's scores alone.
 - Q is produced transposed ([channels, tokens]); scores are
   S_T[keys, q] blocks of [128, 512].
 - Softmax skips the max-subtraction (scores are bounded ~|1.9| after
   the 1/8 scale, applied via the activation's free affine).  exp runs
   on ScalarE over [128, 1024] PSUM groups (one per head per grp).
 - Causal masking multiplies the diagonal score blocks by a 0/1 mask
   after exp; one [128, 1024] multiply per head per diagonal group
   (mask depends only on the key/query offset within the block).
 - V is stored [tokens, 512 ch + 64 ones]; using [V_head | ones] as the
   stationary operand of the P*V matmul makes PSUM rows 0..63 the
   unnormalized output and row 64 the softmax row-sums; normalization
   is 1/s = exp(-ln s) on ScalarE, a DRAM-bounce broadcast, and a
   VectorE multiply.
"""

import sys

if "/opt/trn_rl_repo" not in sys.path:
    sys.path.insert(0, "/opt/trn_rl_repo")

from contextlib import ExitStack

import ml_dtypes
import numpy as np

import concourse.bacc as bacc
import concourse.bass as bass
import concourse.hw_specs as _hw_specs
import concourse.tile as tile
from concourse import mybir
from concourse.bass_utils import run_bass_kernel_spmd

# Bias the activation-table chooser so Exp resolves to the set that also
# contains Ln ("natural_log_exp_and_others"): the kernel interleaves Exp
# (softmax) with Ln (reciprocal via exp(-ln s)), and per-function minimal
# sets would thrash the ~2.7us ACT table load on every switch.
_orig_get_activation_tables = _hw_specs.get_activation_tables


def _patched_activation_tables(arch):
    exp_fn = mybir.ActivationFunctionType.Exp
    out = {}
    for name, fns in _orig_get_activation_tables(arch).items():
        if name != "natural_log_exp_and_others" and exp_fn in fns:
            fns = [f for f in fns if f != exp_fn]
        out[name] = set(fns)
    return out


bacc.get_activation_tables = _patched_activation_tables

B = 4
S = 2048
D = 1024
H = 16
DH = 64

P = 128
DIN_C = D // P        # 8 contraction chunks for the projections
CC = 4                # channel chunks per core (512 / 128)
NHEAD = 8             # heads per core
QT = S // 512         # query tiles of 512
TT = S // 512         # token tiles of 512
VH = 65               # per-head V block: 64 dims + 1 ones column

F32R = mybir.dt.float32r
F32 = mybir.dt.float32
BF16 = mybir.dt.bfloat16

LAST_RESULT = None
_NC_CACHE = None


def build_nc():
    nc = bacc.Bacc()

    xt_d = nc.dram_tensor("xt", [D, S], BF16, kind="ExternalInput")
    wqt_d = nc.dram_tensor("wqt", [D, 512], BF16, kind="ExternalInput")
    wkt_d = nc.dram_tensor("wkt", [D, 512], BF16, kind="ExternalInput")
    wvt_d = nc.dram_tensor("wvt", [D, 512], BF16, kind="ExternalInput")
    wot_d = nc.dram_tensor("wot", [512, D], BF16, kind="ExternalInput")
    mask_d = nc.dram_tensor("mask", [P, 2, 1024], BF16, kind="ExternalInput")
    ones_d = nc.dram_tensor("ones", [P, (S // P) * NHEAD * VH], BF16, kind="ExternalInput")
    yt_d = nc.dram_tensor("yt", [D, S], F32, kind="ExternalOutput")

    xt_v = xt_d[:, :].rearrange("(kc p) t -> p kc t", p=P)
    wq_v = wqt_d[:, :].rearrange("(kc p) c -> p kc c", p=P)
    wk_v = wkt_d[:, :].rearrange("(kc p) c -> p kc c", p=P)
    wv_v = wvt_d[:, :].rearrange("(kc p) c -> p kc c", p=P)
    wot_v = wot_d[:, :].rearrange("(cc p) o -> p cc o", p=P)
    yt_v = yt_d[:, :]

    EXP = mybir.ActivationFunctionType.Exp

    with tile.TileContext(nc) as tc, ExitStack() as ctx:
        singles = ctx.enter_context(tc.tile_pool(name="singles", bufs=1))
        xt_pool = ctx.enter_context(tc.tile_pool(name="xtp", bufs=2))
        qk_pool = ctx.enter_context(tc.tile_pool(name="qkp", bufs=2))
        w_pool = ctx.enter_context(tc.tile_pool(name="wp", bufs=2))
        p_pool = ctx.enter_context(tc.tile_pool(name="pp", bufs=2))
        misc = ctx.enter_context(tc.tile_pool(name="misc", bufs=2))
        yt_pool = ctx.enter_context(tc.tile_pool(name="ytp", bufs=2))
        proj_ps = ctx.enter_context(tc.tile_pool(name="proj_ps", bufs=2, space="PSUM"))
        att_ps = ctx.enter_context(tc.tile_pool(name="att_ps", bufs=1, space="PSUM"))
        dram_pool = ctx.enter_context(tc.tile_pool(name="drp", bufs=2, space="DRAM"))

        v_sb = singles.tile([P, S // P, NHEAD, VH], BF16)
        ot_sb = singles.tile([P, CC, S], BF16)
        wot_sb = singles.tile([P, CC, D], BF16)
        mask_sb = singles.tile([P, 2, 1024], BF16)

        wv_sb = w_pool.tile([P, DIN_C, 512], BF16, tag="wv")

        qk_tiles = {}

        def make_qk(cc):
            wq_sb = w_pool.tile([P, DIN_C, 128], BF16, tag="wq", name=f"wq_{cc}")
            wk_sb = w_pool.tile([P, DIN_C, 128], BF16, tag="wk", name=f"wk_{cc}")
            nc.sync.dma_start(out=wq_sb, in_=wq_v[:, :, cc * 128:(cc + 1) * 128])
            nc.sync.dma_start(out=wk_sb, in_=wk_v[:, :, cc * 128:(cc + 1) * 128])
            qt_sb = qk_pool.tile([P, S], BF16, tag="qt", name=f"qtsb_{cc}")
            # zero-padded K^T: [:, h2] has head h2's 64 dims in partition
            # rows h2*64.. and zeros elsewhere -> score matmuls use full
            # [128, 128] stationaries (no PE tiling-mode switch).
            kt_sb = qk_pool.tile([P, 2, S], BF16, tag="kt", name=f"ktsb_{cc}")
            nc.gpsimd.memset(kt_sb[64:128, 0, :], 0.0)
            nc.gpsimd.memset(kt_sb[0:64, 1, :], 0.0)
            qk_tiles[cc] = (wq_sb, wk_sb, qt_sb, kt_sb)

        def proj_chunks(cc, tt, xt_ready=None):
            """Emit the X-tile DMA now; return compute thunks (one PSUM
            group each) to interleave between attention groups."""
            wq_sb, wk_sb, qt_sb, kt_sb = qk_tiles[cc]
            if xt_ready is not None:
                xt_t = xt_ready
            else:
                xt_t = xt_pool.tile([P, DIN_C, 512], BF16, tag="xt",
                                    name=f"xt_{cc}_{tt}")
                nc.sync.dma_start(out=xt_t[:, 0:4, :],
                                  in_=xt_v[:, 0:4, tt * 512:(tt + 1) * 512])
                nc.sync.dma_start(out=xt_t[:, 4:8, :],
                                  in_=xt_v[:, 4:8, tt * 512:(tt + 1) * 512])
            thunks = []
            if cc == 0:
                for sub in range(4):
                    def vthunk(sub=sub, xt_t=xt_t, tt=tt):
                        vps = proj_ps.tile([P, 512], F32, tag="pp",
                                           name=f"vps_{tt}_{sub}")
                        for kc in range(DIN_C):
                            nc.tensor.matmul(
                                vps,
                                xt_t[:, kc, sub * 128:(sub + 1) * 128],
                                wv_sb[:, kc, :],
                                start=(kc == 0),
                                stop=(kc == DIN_C - 1),
                            )
                        nc.vector.tensor_copy(v_sb[:, tt * 4 + sub, :, 0:64], vps)
                    thunks.append(vthunk)

            def qthunk(xt_t=xt_t, tt=tt, cc=cc, wq_sb=wq_sb, qt_sb=qt_sb):
                qps = proj_ps.tile([P, 512], F32, tag="pp", name=f"qps_{cc}_{tt}")
                for kc in range(DIN_C):
                    nc.tensor.matmul(
                        qps, wq_sb[:, kc, :], xt_t[:, kc, :],
                        start=(kc == 0), stop=(kc == DIN_C - 1),
                    )
                nc.vector.tensor_copy(qt_sb[:, tt * 512:(tt + 1) * 512], qps)

            def kthunk(xt_t=xt_t, tt=tt, cc=cc, wk_sb=wk_sb, kt_sb=kt_sb):
                kps = proj_ps.tile([P, 512], F32, tag="pp", name=f"kps_{cc}_{tt}")
                for kc in range(DIN_C):
                    nc.tensor.matmul(
                        kps, wk_sb[:, kc, :], xt_t[:, kc, :],
                        start=(kc == 0), stop=(kc == DIN_C - 1),
                    )
                nc.vector.tensor_copy(
                    kt_sb[0:64, 0, tt * 512:(tt + 1) * 512], kps[0:64, :])
                nc.vector.tensor_copy(
                    kt_sb[64:128, 1, tt * 512:(tt + 1) * 512], kps[64:128, :])

            # q/k first: their casts gate the next query tile's first score
            # matmuls, while v chunks are only read by later diagonal groups
            return [qthunk, kthunk] + thunks

        def oproj_chunk(tt_o, oc):
            def th():
                ops_o = proj_ps.tile([P, 512], F32, tag="pp",
                                     name=f"ops_o_{tt_o}_{oc}")
                for c2 in range(CC):
                    nc.tensor.matmul(
                        ops_o,
                        wot_sb[:, c2, oc * 128:(oc + 1) * 128],
                        ot_sb[:, c2, tt_o * 512:(tt_o + 1) * 512],
                        start=(c2 == 0),
                        stop=(c2 == CC - 1),
                    )
                y_t = yt_pool.tile([P, 512], F32, tag="yt",
                                   name=f"yt_{tt_o}_{oc}")
                nc.vector.tensor_copy(y_t, ops_o)
                nc.gpsimd.dma_start(
                    out=yt_v[oc * 128:(oc + 1) * 128,
                             tt_o * 512:(tt_o + 1) * 512],
                    in_=y_t,
                )
            return th

        # ---- prologue: X tile on the Sync queue and V weights on the
        # GpSimd queue stream in parallel; Q/K weights follow on Sync. ----
        xt_first = xt_pool.tile([P, DIN_C, 512], BF16, tag="xt", name="xt_0_0")
        nc.sync.dma_start(out=xt_first[:, 0:4, :], in_=xt_v[:, 0:4, 0:512])
        nc.gpsimd.dma_start(out=wv_sb[:, 0:4, :], in_=wv_v[:, 0:4, :])
        nc.sync.dma_start(out=xt_first[:, 4:8, :], in_=xt_v[:, 4:8, 0:512])
        nc.gpsimd.dma_start(out=wv_sb[:, 4:8, :], in_=wv_v[:, 4:8, :])
        make_qk(0)
        pending = proj_chunks(0, 0, xt_ready=xt_first)
        nc.gpsimd.dma_start(out=mask_sb, in_=mask_d[:, :, :])
        # fill v_sb with 1.0 (per token-chunk group so the V copies unblock
        # progressively); the V projection overwrites the data columns,
        # leaving col 64 of each head block as the ones column.
        for q4 in range(4):
            nc.gpsimd.dma_start(
                out=v_sb[:, q4 * 4:(q4 + 1) * 4, :, :],
                in_=ones_d[:, q4 * 4 * NHEAD * VH:(q4 + 1) * 4 * NHEAD * VH],
            )
        nc.gpsimd.dma_start(out=wot_sb, in_=wot_v)
        # V thunks first: wv streams in parallel with xt, while wq/wk queue
        # behind xt on the Sync engine
        for th in pending[2:] + pending[:2]:
            th()

        for cc in range(CC):
            _, _, qt_sb, kt_sb = qk_tiles[cc]
            for qt in range(TT):
                fillers = []
                if qt < TT - 1:
                    fillers += proj_chunks(cc, qt + 1)
                elif cc < CC - 1:
                    make_qk(cc + 1)
                    fillers += proj_chunks(cc + 1, 0)
                if cc == CC - 1 and qt >= 1:
                    fillers += [oproj_chunk(qt - 1, oc) for oc in range(D // P)]

                last_kc = 4 * qt + 3
                n_grps = 2 * qt + 2
                fill_done = 0
                ops = [att_ps.tile([P, 512], F32, tag=f"ops{h2}",
                                   name=f"ops{h2}_{cc}_{qt}")
                       for h2 in range(2)]
                for grp in range(n_grps):
                    sps = [att_ps.tile([P, 1024], F32, tag=f"sps{h2}",
                                       name=f"sps{h2}_{cc}_{qt}_{grp}")
                           for h2 in range(2)]
                    for j in range(2):
                        kc = grp * 2 + j
                        for h2 in range(2):
                            nc.tensor.matmul(
                                sps[h2][:, j * 512:(j + 1) * 512],
                                kt_sb[:, h2, kc * 128:(kc + 1) * 128],
                                qt_sb[:, qt * 512:(qt + 1) * 512],
                                start=True,
                                stop=True,
                            )
                    pts = []
                    for h2 in range(2):
                        p_t = p_pool.tile([P, 1024], BF16, tag=f"p{h2}",
                                          name=f"p{h2}_{cc}_{qt}_{grp}")
                        nc.scalar.activation(p_t, sps[h2], EXP, scale=0.125)
                        pts.append(p_t)
                    if grp >= 2 * qt:  # diagonal groups: causal mask
                        mv = grp - 2 * qt
                        for h2 in range(2):
                            nc.vector.tensor_mul(
                                pts[h2],
                                pts[h2],
                                mask_sb[:, mv, :],
                            )
                    for j in range(2):
                        kc = grp * 2 + j
                        # causal trim: keys in block kc reach only queries
                        # >= kc*128 - qt*512; the earlier P columns are exact
                        # zeros after masking, so skipping them is free
                        qlo = max(0, kc * 128 - qt * 512)
                        for h2 in range(2):
                            nc.tensor.matmul(
                                ops[h2][0:VH, qlo:512],
                                v_sb[:, kc, 2 * cc + h2, 0:VH],
                                pts[h2][:, j * 512 + qlo:(j + 1) * 512],
                                start=(kc == 0),
                                stop=(kc == last_kc),
                                skip_group_check=True,
                            )
                    # spread independent projection work between attention
                    # groups: keeps the PE dense and issues the DVE copies
                    # early so they never gate the next tile's matmuls
                    want = (grp + 1) * len(fillers) // n_grps
                    while fill_done < want:
                        fillers[fill_done]()
                        fill_done += 1
                for h2 in range(2):
                    ops_t = ops[h2]
                    # Move U+sums out of PSUM (frees the opsum bank), then
                    # 1/s on DVE fast-reciprocal, broadcast across the 64
                    # head dims on GpSimd, and scale U on VectorE.
                    u_sb = misc.tile([VH, 512], F32, tag=f"u{h2}",
                                     name=f"u{h2}_{cc}_{qt}")
                    nc.vector.tensor_copy(u_sb, ops_t[0:VH, :])
                    # 1/s = exp(-ln s) on ScalarE (both functions live in the
                    # natural_log_exp_and_others table), then broadcast across
                    # the 64 head dims via a DRAM-bounce DMA.
                    rec_s = misc.tile([VH, 512], F32, tag="recs",
                                      name=f"recs{h2}_{cc}_{qt}")
                    nc.scalar.activation(rec_s[64:65, :], u_sb[64:65, :],
                                         mybir.ActivationFunctionType.Ln)
                    rec_e = misc.tile([VH, 512], F32, tag="rece",
                                      name=f"rece{h2}_{cc}_{qt}")
                    nc.scalar.activation(rec_e[64:65, :], rec_s[64:65, :],
                                         EXP, scale=-1.0)
                    rdram = dram_pool.tile([1, 512], F32, tag="rd",
                                           name=f"rd{h2}_{cc}_{qt}")
                    nc.gpsimd.dma_start(out=rdram, in_=rec_e[64:65, :])
                    rec = misc.tile([64, 512], F32, tag="rec",
                                    name=f"rec{h2}_{cc}_{qt}")
                    rsrc = rdram[0:1, :]
                    nc.gpsimd.dma_start(
                        out=rec,
                        in_=bass.AP(tensor=rsrc.tensor, offset=rsrc.offset,
                                    ap=[[0, 64], [1, 512]]),
                    )
                    nc.vector.tensor_mul(
                        ot_sb[h2 * 64:h2 * 64 + 64, cc,
                              qt * 512:(qt + 1) * 512],
                        u_sb[0:64, :],
                        rec,
                    )
                for th in fillers[fill_done:]:
                    th()

        # tail: last token-tile's output projection
        for oc in range(D // P):
            oproj_chunk(TT - 1, oc)()

    nc.finalize()
    return nc


def _make_mask():
    # variant v covers key blocks 2v,2v+1 (128 keys each) of the diagonal
    # 512-query window: mask[k, v, j*512+q] = (v*256 + j*128 + k <= q)
    keys = (np.arange(2)[None, :, None, None] * 256
            + np.arange(2)[None, None, :, None] * 128
            + np.arange(128)[:, None, None, None])
    qs = np.arange(512)[None, None, None, :]
    return (keys <= qs).astype(np.float32).reshape(128, 2, 1024)


def kernel(X, W_Q, W_K, W_V, W_O):
    global LAST_RESULT, _NC_CACHE
    X = np.asarray(X, dtype=np.float32)
    W_Q = np.asarray(W_Q, dtype=np.float32)
    W_K = np.asarray(W_K, dtype=np.float32)
    W_V = np.asarray(W_V, dtype=np.float32)
    W_O = np.asarray(W_O, dtype=np.float32)

    mask = _make_mask().astype(ml_dtypes.bfloat16)
    in_maps = []
    for c in range(8):
        b, g = c // 2, c % 2
        sl = slice(g * 512, (g + 1) * 512)
        in_maps.append({
            "ones": np.ones((128, 16 * 8 * 65), dtype=ml_dtypes.bfloat16),
            "xt": np.ascontiguousarray(X[b].T).astype(ml_dtypes.bfloat16),
            "wqt": np.ascontiguousarray(W_Q[sl, :].T).astype(ml_dtypes.bfloat16),
            "wkt": np.ascontiguousarray(W_K[sl, :].T).astype(ml_dtypes.bfloat16),
            "wvt": np.ascontiguousarray(W_V[sl, :].T).astype(ml_dtypes.bfloat16),
            "wot": np.ascontiguousarray(W_O[:, sl].T).astype(ml_dtypes.bfloat16),
            "mask": mask,
        })

    if _NC_CACHE is None:
        _NC_CACHE = build_nc()
    res = run_bass_kernel_spmd(_NC_CACHE, in_maps, core_ids=list(range(8)))
    LAST_RESULT = res

    out = np.empty((B, S, D), dtype=np.float32)
    for b in range(B):
        yt = res.results[2 * b]["yt"] + res.results[2 * b + 1]["yt"]
        out[b] = yt.T
    return out



# revision 2
# speedup vs baseline: 1.1595x; 1.1595x over previous
"""Causal multi-head self-attention on 8 Trainium2 NeuronCores.

Problem: X[4, 2048, 1024] fp32, W_Q/W_K/W_V/W_O [1024, 1024] fp32,
16 heads x 64 dims, causal softmax attention + output projection.

Sharding: core c handles batch b = c//2 and head-group g = c%2
(heads g*8..g*8+8, i.e. 512 of the 1024 channels).  Each core computes
its 8 heads' Q/K/V projections, causal attention, and a partial output
projection against W_O[:, g*512:(g+1)*512]; the host sums the two
partial outputs per batch (the "all-reduce after W_O" step).

Device kernel layout notes:
 - Score matmuls are 64-contraction and run as ROW-TILED PAIRS
   (tile 64x128 at row positions 0 and 64): head h2's K^T block
   [64, 128] against its Q rows [64, 512] for both heads of a channel
   chunk execute concurrently in the two halves of the PE array.
 - Q/K are produced transposed ([channels, tokens]); K needs no zero
   padding: kt[h2*64:(h2+1)*64, t] holds head h2's dims.
 - scores land in ONE psum tile sps[128 keys, 2 heads, 1024
   (2 key-chunks x 512 q)]; a single exp ACT covers both heads
   (halves the per-instruction ACT overhead).
 - Softmax skips the max-subtraction (scores are bounded ~|1.9| after
   the 1/8 scale, applied via the activation's free affine).
 - Causal masking multiplies the diagonal score blocks by a 0/1 mask
   after exp; exp/mask skip the fully-masked leading region of the
   second diagonal group.
 - V is stored [tokens, 512 ch + 64 ones]; using [V_head | ones] as the
   stationary operand of the P*V matmul makes PSUM rows 0..63 the
   unnormalized output and row 64 the softmax row-sums; normalization
   is 1/s = exp(-ln s) on ScalarE (both heads' sums batched into one
   [1, 2, 512] Ln+Exp pair), a DRAM-bounce broadcast, and VectorE
   multiplies.
 - The P*V matmuls for group g are issued AFTER group g+1's score
   matmuls (one-group software-pipeline skew) so the in-order tensor
   queue has score+filler work to run while group g+1's exp is on the
   scalar engine.
 - Projection/output-projection matmuls are spread between attention
   groups as fillers; output is stored bf16 (host accumulates in f32).
"""

import sys

if "/opt/trn_rl_repo" not in sys.path:
    sys.path.insert(0, "/opt/trn_rl_repo")

from contextlib import ExitStack

import ml_dtypes
import numpy as np

import concourse.bacc as bacc
import concourse.bass as bass
import concourse.hw_specs as _hw_specs
import concourse.tile as tile
from concourse import mybir
from concourse.bass_utils import run_bass_kernel_spmd

# Bias the activation-table chooser so Exp resolves to the set that also
# contains Ln ("natural_log_exp_and_others"): the kernel interleaves Exp
# (softmax) with Ln (reciprocal via exp(-ln s)), and per-function minimal
# sets would thrash the ~2.7us ACT table load on every switch.
_orig_get_activation_tables = _hw_specs.get_activation_tables


def _patched_activation_tables(arch):
    exp_fn = mybir.ActivationFunctionType.Exp
    out = {}
    for name, fns in _orig_get_activation_tables(arch).items():
        if name != "natural_log_exp_and_others" and exp_fn in fns:
            fns = [f for f in fns if f != exp_fn]
        out[name] = set(fns)
    return out


bacc.get_activation_tables = _patched_activation_tables

B = 4
S = 2048
D = 1024
H = 16
DH = 64

P = 128
DIN_C = D // P        # 8 contraction chunks for the projections
CC = 4                # channel chunks per core (512 / 128)
NHEAD = 8             # heads per core
QT = S // 512         # query tiles of 512
TT = S // 512         # token tiles of 512
VH = 65               # per-head V block: 64 dims + 1 ones column

F32R = mybir.dt.float32r
F32 = mybir.dt.float32
BF16 = mybir.dt.bfloat16

LAST_RESULT = None
_NC_CACHE = None


def build_nc():
    nc = bacc.Bacc()

    xt_d = nc.dram_tensor("xt", [D, S], BF16, kind="ExternalInput")
    wqt_d = nc.dram_tensor("wqt", [D, 512], BF16, kind="ExternalInput")
    wkt_d = nc.dram_tensor("wkt", [D, 512], BF16, kind="ExternalInput")
    wvt_d = nc.dram_tensor("wvt", [D, 512], BF16, kind="ExternalInput")
    wot_d = nc.dram_tensor("wot", [512, D], BF16, kind="ExternalInput")
    mask_d = nc.dram_tensor("mask", [P, 2, 1024], BF16, kind="ExternalInput")
    yt_d = nc.dram_tensor("yt", [D, S], BF16, kind="ExternalOutput")

    xt_v = xt_d[:, :].rearrange("(kc p) t -> p kc t", p=P)
    wq_v = wqt_d[:, :].rearrange("(kc p) c -> p kc c", p=P)
    wk_v = wkt_d[:, :].rearrange("(kc p) c -> p kc c", p=P)
    wv_v = wvt_d[:, :].rearrange("(kc p) c -> p kc c", p=P)
    wot_v = wot_d[:, :].rearrange("(cc p) o -> p cc o", p=P)
    yt_v = yt_d[:, :]

    EXP = mybir.ActivationFunctionType.Exp

    with tile.TileContext(nc) as tc, ExitStack() as ctx:
        singles = ctx.enter_context(tc.tile_pool(name="singles", bufs=1))
        xt_pool = ctx.enter_context(tc.tile_pool(name="xtp", bufs=2))
        qk_pool = ctx.enter_context(tc.tile_pool(name="qkp", bufs=2))
        w_pool = ctx.enter_context(tc.tile_pool(name="wp", bufs=2))
        p_pool = ctx.enter_context(tc.tile_pool(name="pp", bufs=2))
        misc = ctx.enter_context(tc.tile_pool(name="misc", bufs=2))
        yt_pool = ctx.enter_context(tc.tile_pool(name="ytp", bufs=2))
        proj_ps = ctx.enter_context(tc.tile_pool(name="proj_ps", bufs=2, space="PSUM"))
        att_ps = ctx.enter_context(tc.tile_pool(name="att_ps", bufs=1, space="PSUM"))
        dram_pool = ctx.enter_context(tc.tile_pool(name="drp", bufs=2, space="DRAM"))

        v_sb = singles.tile([P, S // P, NHEAD, VH], BF16)
        ot_sb = singles.tile([P, CC, S], BF16)
        wot_sb = singles.tile([P, CC, D], BF16)
        mask_sb = singles.tile([P, 2, 1024], BF16)

        wv_sb = w_pool.tile([P, DIN_C, 512], BF16, tag="wv")

        qk_tiles = {}

        def make_qk(cc):
            wq_sb = w_pool.tile([P, DIN_C, 128], BF16, tag="wq", name=f"wq_{cc}")
            wk_sb = w_pool.tile([P, DIN_C, 128], BF16, tag="wk", name=f"wk_{cc}")
            nc.sync.dma_start(out=wq_sb, in_=wq_v[:, :, cc * 128:(cc + 1) * 128])
            nc.sync.dma_start(out=wk_sb, in_=wk_v[:, :, cc * 128:(cc + 1) * 128])
            qt_sb = qk_pool.tile([P, S], BF16, tag="qt", name=f"qtsb_{cc}")
            # K^T stacked like Q: head h2's 64 dims live in partition rows
            # h2*64..h2*64+63 -> score matmuls are row-tiled 64x128 pairs.
            kt_sb = qk_pool.tile([P, S], BF16, tag="kt", name=f"ktsb_{cc}")
            qk_tiles[cc] = (wq_sb, wk_sb, qt_sb, kt_sb)

        def proj_chunks(cc, tt, xt_ready=None):
            """Emit the X-tile DMA now; return compute thunks (one PSUM
            group each) to interleave between attention groups."""
            wq_sb, wk_sb, qt_sb, kt_sb = qk_tiles[cc]
            if xt_ready is not None:
                xt_t = xt_ready
            else:
                xt_t = xt_pool.tile([P, DIN_C, 512], BF16, tag="xt",
                                    name=f"xt_{cc}_{tt}")
                nc.sync.dma_start(out=xt_t[:, 0:4, :],
                                  in_=xt_v[:, 0:4, tt * 512:(tt + 1) * 512])
                nc.sync.dma_start(out=xt_t[:, 4:8, :],
                                  in_=xt_v[:, 4:8, tt * 512:(tt + 1) * 512])
            thunks = []
            if cc == 0:
                for sub in range(4):
                    def vthunk(sub=sub, xt_t=xt_t, tt=tt):
                        vps = proj_ps.tile([P, 512], F32, tag="pp",
                                           name=f"vps_{tt}_{sub}")
                        for kc in range(DIN_C):
                            nc.tensor.matmul(
                                vps,
                                xt_t[:, kc, sub * 128:(sub + 1) * 128],
                                wv_sb[:, kc, :],
                                start=(kc == 0),
                                stop=(kc == DIN_C - 1),
                            )
                        nc.vector.tensor_copy(v_sb[:, tt * 4 + sub, :, 0:64], vps)
                    thunks.append(vthunk)

            def qthunk(xt_t=xt_t, tt=tt, cc=cc, wq_sb=wq_sb, qt_sb=qt_sb):
                qps = proj_ps.tile([P, 512], F32, tag="pp", name=f"qps_{cc}_{tt}")
                for kc in range(DIN_C):
                    nc.tensor.matmul(
                        qps, wq_sb[:, kc, :], xt_t[:, kc, :],
                        start=(kc == 0), stop=(kc == DIN_C - 1),
                    )
                nc.vector.tensor_copy(qt_sb[:, tt * 512:(tt + 1) * 512], qps)

            def kthunk(xt_t=xt_t, tt=tt, cc=cc, wk_sb=wk_sb, kt_sb=kt_sb):
                kps = proj_ps.tile([P, 512], F32, tag="pp", name=f"kps_{cc}_{tt}")
                for kc in range(DIN_C):
                    nc.tensor.matmul(
                        kps, wk_sb[:, kc, :], xt_t[:, kc, :],
                        start=(kc == 0), stop=(kc == DIN_C - 1),
                    )
                nc.vector.tensor_copy(kt_sb[:, tt * 512:(tt + 1) * 512], kps)

            # q/k first: their casts gate the next query tile's first score
            # matmuls, while v chunks are only read by later diagonal groups
            return [qthunk, kthunk] + thunks

        def oproj_chunk(tt_o, oc):
            def th():
                ops_o = proj_ps.tile([P, 512], F32, tag="pp",
                                     name=f"ops_o_{tt_o}_{oc}")
                for c2 in range(CC):
                    nc.tensor.matmul(
                        ops_o,
                        wot_sb[:, c2, oc * 128:(oc + 1) * 128],
                        ot_sb[:, c2, tt_o * 512:(tt_o + 1) * 512],
                        start=(c2 == 0),
                        stop=(c2 == CC - 1),
                    )
                y_t = yt_pool.tile([P, 512], BF16, tag="yt",
                                   name=f"yt_{tt_o}_{oc}")
                nc.vector.tensor_copy(y_t, ops_o)
                nc.gpsimd.dma_start(
                    out=yt_v[oc * 128:(oc + 1) * 128,
                             tt_o * 512:(tt_o + 1) * 512],
                    in_=y_t,
                )
            return th

        # ---- prologue: X tile on the Sync queue and V weights on the
        # GpSimd queue stream in parallel; Q/K weights follow on Sync. ----
        xt_first = xt_pool.tile([P, DIN_C, 512], BF16, tag="xt", name="xt_0_0")
        nc.sync.dma_start(out=xt_first[:, 0:4, :], in_=xt_v[:, 0:4, 0:512])
        nc.gpsimd.dma_start(out=wv_sb[:, 0:4, :], in_=wv_v[:, 0:4, :])
        nc.sync.dma_start(out=xt_first[:, 4:8, :], in_=xt_v[:, 4:8, 0:512])
        nc.gpsimd.dma_start(out=wv_sb[:, 4:8, :], in_=wv_v[:, 4:8, :])
        make_qk(0)
        pending = proj_chunks(0, 0, xt_ready=xt_first)
        nc.gpsimd.dma_start(out=mask_sb, in_=mask_d[:, :, :])
        # the V-projection copies fill the data columns; only col 64 of each
        # head block (the ones column for the P*V row-sum trick) is set here
        # (per token-chunk group so the V copies unblock progressively).
        for q4 in range(4):
            nc.gpsimd.memset(v_sb[:, q4 * 4:(q4 + 1) * 4, :, 64:65], 1.0)
        nc.gpsimd.dma_start(out=wot_sb, in_=wot_v)
        # V thunks first: wv streams in parallel with xt, while wq/wk queue
        # behind xt on the Sync engine
        for th in pending[2:] + pending[:2]:
            th()

        for cc in range(CC):
            _, _, qt_sb, kt_sb = qk_tiles[cc]
            for qt in range(TT):
                fillers = []
                if qt < TT - 1:
                    fillers += proj_chunks(cc, qt + 1)
                elif cc < CC - 1:
                    make_qk(cc + 1)
                    fillers += proj_chunks(cc + 1, 0)
                if cc == CC - 1 and qt >= 1:
                    fillers += [oproj_chunk(qt - 1, oc) for oc in range(D // P)]

                last_kc = 4 * qt + 3
                n_grps = 2 * qt + 2
                fill_done = 0
                ops = [att_ps.tile([P, 512], F32, tag=f"ops{h2}",
                                   name=f"ops{h2}_{cc}_{qt}")
                       for h2 in range(2)]
                pv_prev = None
                for grp in range(n_grps):
                    sps = att_ps.tile([P, 2, 1024], F32, tag="sps",
                                      name=f"sps_{cc}_{qt}_{grp}")
                    for j in range(2):
                        kc = grp * 2 + j
                        for h2 in range(2):
                            # 64-contraction row-tiled pair: h2=0 in array
                            # rows 0-63, h2=1 in rows 64-127, concurrent.
                            nc.tensor.matmul(
                                sps[:, h2, j * 512:(j + 1) * 512],
                                kt_sb[h2 * 64:(h2 + 1) * 64,
                                      kc * 128:(kc + 1) * 128],
                                qt_sb[h2 * 64:(h2 + 1) * 64,
                                      qt * 512:(qt + 1) * 512],
                                start=True,
                                stop=True,
                            )
                    p_t = p_pool.tile([P, 2, 1024], BF16, tag="p",
                                      name=f"p_{cc}_{qt}_{grp}")
                    # leading fully-masked columns of the 2nd diagonal group
                    # are never read by the trimmed P*V matmuls: skip them
                    elo = 256 if grp == 2 * qt + 1 else 0
                    nc.scalar.activation(p_t[:, :, elo:], sps[:, :, elo:],
                                         EXP, scale=0.125)
                    if grp >= 2 * qt:  # diagonal groups: causal mask
                        mv = grp - 2 * qt
                        for h2 in range(2):
                            nc.vector.tensor_mul(
                                p_t[:, h2, elo:],
                                p_t[:, h2, elo:],
                                mask_sb[:, mv, elo:],
                            )

                    def pv_thunk(grp=grp, p_t=p_t, cc=cc, qt=qt,
                                 last_kc=last_kc):
                        for j in range(2):
                            kc = grp * 2 + j
                            # causal trim: keys in block kc reach only
                            # queries >= kc*128 - qt*512; earlier columns
                            # are exact zeros after masking, skip them
                            qlo = max(0, kc * 128 - qt * 512)
                            for h2 in range(2):
                                nc.tensor.matmul(
                                    ops[h2][0:VH, qlo:512],
                                    v_sb[:, kc, 2 * cc + h2, 0:VH],
                                    p_t[:, h2, j * 512 + qlo:(j + 1) * 512],
                                    start=(kc == 0),
                                    stop=(kc == last_kc),
                                    skip_group_check=True,
                                )

                    # one-group skew: run the PREVIOUS group's P*V now, so
                    # the tensor queue isn't head-blocked on this group's
                    # exp; fillers (projection work) pad the rest.
                    if pv_prev is not None:
                        pv_prev()
                    pv_prev = pv_thunk
                    want = (grp + 1) * len(fillers) // n_grps
                    while fill_done < want:
                        fillers[fill_done]()
                        fill_done += 1
                pv_prev()
                for th in fillers[fill_done:]:
                    th()

                # ---- normalization: U / s with s from the ones column ----
                u_sb = misc.tile([VH, 2, 512], F32, tag="u",
                                 name=f"u_{cc}_{qt}")
                for h2 in range(2):
                    nc.vector.tensor_copy(u_sb[:, h2, :], ops[h2][0:VH, :])
                # 1/s = exp(-ln s) on ScalarE, both heads in one [1,2,512]
                # pair (Ln and Exp share the natural_log_exp table), then a
                # DRAM-bounce DMA broadcasts across the 64 head dims.
                rec_s = misc.tile([VH, 2, 512], F32, tag="recs",
                                  name=f"recs_{cc}_{qt}")
                nc.scalar.activation(rec_s[64:65, :, :], u_sb[64:65, :, :],
                                     mybir.ActivationFunctionType.Ln)
                rec_e = misc.tile([VH, 2, 512], F32, tag="rece",
                                  name=f"rece_{cc}_{qt}")
                nc.scalar.activation(rec_e[64:65, :, :], rec_s[64:65, :, :],
                                     EXP, scale=-1.0)
                rdram = dram_pool.tile([1, 2, 512], F32, tag="rd",
                                       name=f"rd_{cc}_{qt}")
                nc.gpsimd.dma_start(out=rdram, in_=rec_e[64:65, :, :])
                rec = misc.tile([64, 2, 512], F32, tag="rec",
                                name=f"rec_{cc}_{qt}")
                for h2 in range(2):
                    rsrc = rdram[0:1, h2, :]
                    nc.gpsimd.dma_start(
                        out=rec[:, h2, :],
                        in_=bass.AP(tensor=rsrc.tensor, offset=rsrc.offset,
                                    ap=[[0, 64], [1, 512]]),
                    )
                for h2 in range(2):
                    nc.vector.tensor_mul(
                        ot_sb[h2 * 64:h2 * 64 + 64, cc,
                              qt * 512:(qt + 1) * 512],
                        u_sb[0:64, h2, :],
                        rec[:, h2, :],
                    )

        # tail: last token-tile's output projection
        for oc in range(D // P):
            oproj_chunk(TT - 1, oc)()

    nc.finalize()
    return nc


def _make_mask():
    # variant v covers key blocks 2v,2v+1 (128 keys each) of the diagonal
    # 512-query window: mask[k, v, j*512+q] = (v*256 + j*128 + k <= q)
    keys = (np.arange(2)[None, :, None, None] * 256
            + np.arange(2)[None, None, :, None] * 128
            + np.arange(128)[:, None, None, None])
    qs = np.arange(512)[None, None, None, :]
    return (keys <= qs).astype(np.float32).reshape(128, 2, 1024)


def kernel(X, W_Q, W_K, W_V, W_O):
    global LAST_RESULT, _NC_CACHE
    X = np.asarray(X, dtype=np.float32)
    W_Q = np.asarray(W_Q, dtype=np.float32)
    W_K = np.asarray(W_K, dtype=np.float32)
    W_V = np.asarray(W_V, dtype=np.float32)
    W_O = np.asarray(W_O, dtype=np.float32)

    mask = _make_mask().astype(ml_dtypes.bfloat16)
    in_maps = []
    for c in range(8):
        b, g = c // 2, c % 2
        sl = slice(g * 512, (g + 1) * 512)
        in_maps.append({
            "xt": np.ascontiguousarray(X[b].T).astype(ml_dtypes.bfloat16),
            "wqt": np.ascontiguousarray(W_Q[sl, :].T).astype(ml_dtypes.bfloat16),
            "wkt": np.ascontiguousarray(W_K[sl, :].T).astype(ml_dtypes.bfloat16),
            "wvt": np.ascontiguousarray(W_V[sl, :].T).astype(ml_dtypes.bfloat16),
            "wot": np.ascontiguousarray(W_O[:, sl].T).astype(ml_dtypes.bfloat16),
            "mask": mask,
        })

    if _NC_CACHE is None:
        _NC_CACHE = build_nc()
    res = run_bass_kernel_spmd(_NC_CACHE, in_maps, core_ids=list(range(8)))
    LAST_RESULT = res

    out = np.empty((B, S, D), dtype=np.float32)
    for b in range(B):
        yt = (res.results[2 * b]["yt"].astype(np.float32)
              + res.results[2 * b + 1]["yt"].astype(np.float32))
        out[b] = yt.T
    return out
